# revision 8
# baseline (speedup 1.0000x reference)
"""Varlen causal GQA attention on 8 TRN2 NeuronCores.

Problem: 32 q heads, 8 kv heads, head_dim 128, ragged batch (cu_seqlens),
f32. Sharded by KV-head group: core c owns kv head c and q heads
4c..4c+3 — fully data-independent across cores, no collectives.

Per core, for each of its 4 q heads, blockwise causal attention per
sequence with k-blocks in the outer loop (stationary reuse across the
q-groups of a sequence):
    S^T[k, q] = (K_j)^T.T @ Q^T          (bf16 matmul, d contracted)
    S^T += causal mask on diagonal block (DVE, fp32 in PSUM)
    P^T = exp(S^T * scale)               (ScalarE, bf16 out)
    O^T[d, q] += V_j @ P^T               (lhsT = V_j natural [k, d])
    sums[1, q] += ones.T @ P^T
Host does all transposes (Q^T/K^T in, O^T -> O out), the bf16 input
conversion, and the final softmax division, so the device executes only
matmuls, exp, mask adds, and PSUM->SBUF copies.
"""

import math
import os
import sys

sys.path.insert(0, "/opt/trn_rl_repo")

import ml_dtypes
import numpy as np

NUM_HEADS = 32
NUM_KV_HEADS = 8
HEAD_DIM = 128
HEADS_PER_CORE = NUM_HEADS // NUM_KV_HEADS  # 4
N_CORES = 8
BLK = 128
GROUP = 512
SCALE = 1.0 / math.sqrt(HEAD_DIM)

_GRAPH_CACHE = {}


def _build_graph(seq_blocks):
    """Build the SPMD Bacc graph for padded per-seq block counts."""
    from concourse import bacc
    import concourse.mybir as mybir
    from concourse.tile import TileContext

    f32 = mybir.dt.float32
    bf16 = mybir.dt.bfloat16
    T = sum(seq_blocks) * BLK
    n_blocks_total = T // BLK

    nc = bacc.Bacc("TRN2", target_bir_lowering=False, debug=False,
                   num_devices=N_CORES)

    qT_ext = [
        nc.declare_dram_parameter(f"qT{h}", [BLK, T], bf16, isOutput=False)
        for h in range(HEADS_PER_CORE)
    ]
    kT_ext = nc.declare_dram_parameter("kT", [BLK, T], bf16, isOutput=False)
    v_ext = nc.declare_dram_parameter("v", [T, HEAD_DIM], bf16, isOutput=False)
    mask_ext = nc.declare_dram_parameter("mask", [BLK, BLK], f32, isOutput=False)
    oT_ext = [
        nc.declare_dram_parameter(f"oT{h}", [BLK, T], f32, isOutput=True)
        for h in range(HEADS_PER_CORE)
    ]
    sums_ext = [
        nc.declare_dram_parameter(f"sums{h}", [1, T], f32, isOutput=True)
        for h in range(HEADS_PER_CORE)
    ]

    with TileContext(nc) as tc:
        with (
            tc.tile_pool(name="persist", bufs=1) as persist,
            tc.tile_pool(name="qr", bufs=2) as qr_pool,
            tc.tile_pool(name="ot", bufs=2) as ot_pool,
            tc.tile_pool(name="p", bufs=4) as p_pool,
            tc.tile_pool(name="ps_s", bufs=2, space="PSUM") as ps_s,
            tc.tile_pool(name="ps_o", bufs=3, space="PSUM") as ps_o,
            tc.tile_pool(name="ps_sum", bufs=3, space="PSUM") as ps_sum,
        ):
            kT_sb = persist.tile([BLK, T], bf16)
            v_sb = persist.tile([BLK, n_blocks_total, HEAD_DIM], bf16)
            mask_sb = persist.tile([BLK, BLK], f32)
            nc.sync.dma_start(kT_sb[:], kT_ext[:])
            nc.sync.dma_start(
                v_sb[:], v_ext[:].rearrange("(j p) d -> p j d", p=BLK)
            )
            nc.sync.dma_start(mask_sb[:], mask_ext[:])

            ones_f = persist.tile([BLK, 1], f32)
            nc.vector.memset(ones_f[:], 1.0)
            ones_b = persist.tile([BLK, 1], bf16)
            nc.vector.tensor_copy(ones_b[:], ones_f[:])

            for h in range(HEADS_PER_CORE):
                qT_sb = qr_pool.tile([BLK, T], bf16, tag="q")
                nc.sync.dma_start(qT_sb[:], qT_ext[h][:])
                ot_stage = ot_pool.tile([BLK, T], f32, tag="ot")
                sums_stage = ot_pool.tile([1, T], f32, tag="sums")

                seq_off = 0
                for nblk in seq_blocks:
                    Ls = nblk * BLK
                    n_groups = (Ls + GROUP - 1) // GROUP
                    # per-group accumulators live across the whole j loop
                    oT_ps = [
                        ps_o.tile([BLK, GROUP], f32, tag="ot_ps", name="oT_ps")
                        for _ in range(n_groups)
                    ]
                    sums_ps = [
                        ps_sum.tile([1, GROUP], f32, tag="sums_ps", name="sums_ps")
                        for _ in range(n_groups)
                    ]
                    g_started = [False] * n_groups
                    for j in range(nblk):
                        kj = kT_sb[:, seq_off + j * BLK : seq_off + (j + 1) * BLK]
                        vj = v_sb[:, seq_off // BLK + j, :]
                        for g in range(n_groups):
                            Q0 = g * GROUP
                            W = min(GROUP, Ls - Q0)
                            cs = max(0, BLK * j - Q0)
                            if cs >= W:
                                continue  # k block entirely after this group
                            N = W - cs
                            s_ps = ps_s.tile([BLK, GROUP], f32, tag="s_ps")
                            nc.tensor.matmul(
                                s_ps[:, :N],
                                kj,
                                qT_sb[:, seq_off + Q0 + cs : seq_off + Q0 + cs + N],
                                start=True,
                                stop=True,
                            )
                            if BLK * j >= Q0:  # diagonal block: causal mask
                                nc.vector.tensor_add(
                                    s_ps[:, :BLK], s_ps[:, :BLK], mask_sb[:]
                                )
                            p_b = p_pool.tile([BLK, GROUP], bf16, tag="p")
                            nc.scalar.activation(
                                p_b[:, :N],
                                s_ps[:, :N],
                                mybir.ActivationFunctionType.Exp,
                                scale=SCALE,
                            )
                            last = j == (Q0 + W) // BLK - 1
                            nc.tensor.matmul(
                                oT_ps[g][:, cs : cs + N],
                                vj,
                                p_b[:, :N],
                                start=not g_started[g],
                                stop=last,
                            )
                            nc.tensor.matmul(
                                sums_ps[g][:, cs : cs + N],
                                ones_b[:],
                                p_b[:, :N],
                                start=not g_started[g],
                                stop=last,
                            )
                            g_started[g] = True
                    for g in range(n_groups):
                        Q0 = g * GROUP
                        W = min(GROUP, Ls - Q0)
                        nc.vector.tensor_copy(
                            ot_stage[:, seq_off + Q0 : seq_off + Q0 + W],
                            oT_ps[g][:, :W],
                        )
                        nc.vector.tensor_copy(
                            sums_stage[:, seq_off + Q0 : seq_off + Q0 + W],
                            sums_ps[g][:, :W],
                        )
                    seq_off += Ls

                nc.sync.dma_start(oT_ext[h][:], ot_stage[:])
                nc.sync.dma_start(sums_ext[h][:], sums_stage[:])

    nc.finalize()
    return nc


def _install_ntff_hook():
    """Shim antenv.axon_hooks (absent in this container) so trace=True can
    reach the terminal's NRT profiler via libaxon_pjrt.so ctypes."""
    import types

    if "antenv.axon_hooks" in sys.modules:
        return
    import antenv
    from concourse import bass_utils

    mod = types.ModuleType("antenv.axon_hooks")
    state = {"hook": None}
    mod.set_axon_ntff_profile_hook = lambda h: state.__setitem__("hook", h)
    mod.get_axon_ntff_profile_hook = lambda: state["hook"]
    sys.modules["antenv.axon_hooks"] = mod
    antenv.axon_hooks = mod
    bass_utils.upload_artifacts = lambda tmpdir: tmpdir  # zero-egress container
    try:
        if "/root/.axon_site" not in sys.path:
            sys.path.insert(0, "/root/.axon_site")
        from trn_agent_boot.trn_boot import _ntff_profile_via_ctypes

        mod.set_axon_ntff_profile_hook(
            _ntff_profile_via_ctypes("/opt/axon/libaxon_pjrt.so")
        )
    except Exception:
        pass


def kernel(q, k, v, cu_seqlens, max_seqlen):
    from concourse import bass_utils

    q = np.asarray(q, dtype=np.float32)
    k = np.asarray(k, dtype=np.float32)
    v = np.asarray(v, dtype=np.float32)
    cu = np.asarray(cu_seqlens, dtype=np.int64)
    T_host = q.shape[0]
    lengths = np.diff(cu).astype(np.int64)
    nblocks = [int((L + BLK - 1) // BLK) for L in lengths]
    T_pad = sum(nblocks) * BLK

    # host -> padded device token index map (valid tokens only)
    dev_idx = np.zeros(T_host, dtype=np.int64)
    pad_off = 0
    for s, L in enumerate(lengths):
        L = int(L)
        dev_idx[cu[s] : cu[s] + L] = pad_off + np.arange(L)
        pad_off += nblocks[s] * BLK

    bf16 = ml_dtypes.bfloat16
    qp = np.zeros((T_pad, NUM_HEADS * HEAD_DIM), bf16)
    kp = np.zeros((T_pad, NUM_KV_HEADS * HEAD_DIM), bf16)
    vp = np.zeros((T_pad, NUM_KV_HEADS * HEAD_DIM), bf16)
    qp[dev_idx] = q.astype(bf16)
    kp[dev_idx] = k.astype(bf16)
    vp[dev_idx] = v.astype(bf16)

    mask = np.where(
        np.arange(BLK)[:, None] <= np.arange(BLK)[None, :], 0.0, -1e30
    ).astype(np.float32)

    key = tuple(nblocks)
    if key not in _GRAPH_CACHE:
        _GRAPH_CACHE[key] = _build_graph(key)
    nc = _GRAPH_CACHE[key]

    in_maps = []
    for c in range(N_CORES):
        m = {"mask": mask}
        m["kT"] = np.ascontiguousarray(kp[:, c * HEAD_DIM : (c + 1) * HEAD_DIM].T)
        m["v"] = np.ascontiguousarray(vp[:, c * HEAD_DIM : (c + 1) * HEAD_DIM])
        for h in range(HEADS_PER_CORE):
            gh = c * HEADS_PER_CORE + h
            m[f"qT{h}"] = np.ascontiguousarray(
                qp[:, gh * HEAD_DIM : (gh + 1) * HEAD_DIM].T
            )
        in_maps.append(m)

    trace = bool(os.environ.get("BASS_TRACE"))
    if trace:
        _install_ntff_hook()
    res = bass_utils.run_bass_kernel_spmd(
        nc, in_maps, core_ids=list(range(N_CORES)), trace=trace
    )
    if trace and res.exec_time_ns is not None:
        print(f"HW exec time: {res.exec_time_ns} ns")
        if res.instructions_and_trace is not None:
            print(f"trace: {res.instructions_and_trace[1]}")

    out = np.empty((T_host, NUM_HEADS * HEAD_DIM), np.float32)
    for c in range(N_CORES):
        r = res.results[c]
        for h in range(HEADS_PER_CORE):
            gh = c * HEADS_PER_CORE + h
            oT = r[f"oT{h}"]  # [128, T_pad] unnormalized
            sums = r[f"sums{h}"][0]  # [T_pad]
            o = (oT[:, dev_idx] / sums[dev_idx][None, :]).T  # [T_host, 128]
            out[:, gh * HEAD_DIM : (gh + 1) * HEAD_DIM] = o
    return out


# revision 12
# speedup vs baseline: 1.1984x; 1.1984x over previous
"""Varlen causal GQA attention on 8 TRN2 NeuronCores.

Problem: 32 q heads, 8 kv heads, head_dim 128, ragged batch (cu_seqlens),
f32. Sharded by KV-head group: core c owns kv head c and q heads
4c..4c+3 — fully data-independent across cores, no collectives.

Per core, for each of its 4 q heads, blockwise causal attention per
sequence with k-blocks in the outer loop (stationary reuse across the
q-groups of a sequence):
    S^T[k, q] = (K_j)^T.T @ Q^T          (bf16 matmul, d contracted)
    S^T += causal mask on diagonal block (DVE, fp32 in PSUM)
    P^T = exp(S^T * scale)               (ScalarE, bf16 out)
    O^T[d, q] += V_j @ P^T               (lhsT = V_j natural [k, d])
    sums[1, q] += ones.T @ P^T
Host does all transposes (Q^T/K^T in, O^T -> O out), the bf16 input
conversion, and the final softmax division, so the device executes only
matmuls, exp, mask adds, and PSUM->SBUF copies.
"""

import math
import os
import sys

sys.path.insert(0, "/opt/trn_rl_repo")

import ml_dtypes
import numpy as np

NUM_HEADS = 32
NUM_KV_HEADS = 8
HEAD_DIM = 128
HEADS_PER_CORE = NUM_HEADS // NUM_KV_HEADS  # 4
N_CORES = 8
BLK = 128
GROUP = 512
SCALE = 1.0 / math.sqrt(HEAD_DIM)

_GRAPH_CACHE = {}


def _build_graph(seq_blocks):
    """Build the SPMD Bacc graph for padded per-seq block counts."""
    from concourse import bacc
    import concourse.mybir as mybir
    from concourse.tile import TileContext

    f32 = mybir.dt.float32
    bf16 = mybir.dt.bfloat16
    T = sum(seq_blocks) * BLK
    n_blocks_total = T // BLK

    nc = bacc.Bacc("TRN2", target_bir_lowering=False, debug=False,
                   num_devices=N_CORES)

    qT_ext = [
        nc.declare_dram_parameter(f"qT{h}", [BLK, T], bf16, isOutput=False)
        for h in range(HEADS_PER_CORE)
    ]
    kT_ext = nc.declare_dram_parameter("kT", [BLK, T], bf16, isOutput=False)
    v_ext = nc.declare_dram_parameter("v", [T, HEAD_DIM], bf16, isOutput=False)
    mask_ext = nc.declare_dram_parameter("mask", [BLK, BLK], f32, isOutput=False)
    oT_ext = [
        nc.declare_dram_parameter(f"oT{h}", [BLK, T], f32, isOutput=True)
        for h in range(HEADS_PER_CORE)
    ]
    sums_ext = [
        nc.declare_dram_parameter(f"sums{h}", [1, T], f32, isOutput=True)
        for h in range(HEADS_PER_CORE)
    ]

    with TileContext(nc) as tc:
        with (
            tc.tile_pool(name="persist", bufs=1) as persist,
            tc.tile_pool(name="qr", bufs=2) as qr_pool,
            tc.tile_pool(name="ot", bufs=2) as ot_pool,
            tc.tile_pool(name="p", bufs=6) as p_pool,
            tc.tile_pool(name="ps_s", bufs=3, space="PSUM") as ps_s,
            tc.tile_pool(name="ps_o", bufs=3, space="PSUM") as ps_o,
            tc.tile_pool(name="ps_sum", bufs=2, space="PSUM") as ps_sum,
        ):
            kT_sb = persist.tile([BLK, T], bf16)
            v_sb = persist.tile([BLK, n_blocks_total, HEAD_DIM], bf16)
            mask_sb = persist.tile([BLK, BLK], f32)
            nc.sync.dma_start(mask_sb[:], mask_ext[:])
            # chunk k/v loads per sequence so head-0/seq-0 compute starts early
            v_re = v_ext[:].rearrange("(j p) d -> p j d", p=BLK)
            off = 0
            for nblk in seq_blocks:
                c0, c1 = off * BLK, (off + nblk) * BLK
                nc.sync.dma_start(kT_sb[:, c0:c1], kT_ext[:, c0:c1])
                nc.sync.dma_start(
                    v_sb[:, off : off + nblk, :], v_re[:, off : off + nblk, :]
                )
                off += nblk

            ones_f = persist.tile([BLK, 1], f32)
            nc.vector.memset(ones_f[:], 1.0)
            ones_b = persist.tile([BLK, 1], bf16)
            nc.vector.tensor_copy(ones_b[:], ones_f[:])

            for h in range(HEADS_PER_CORE):
                qT_sb = qr_pool.tile([BLK, T], bf16, tag="q")
                off = 0
                for nblk in seq_blocks:
                    c0, c1 = off * BLK, (off + nblk) * BLK
                    nc.sync.dma_start(qT_sb[:, c0:c1], qT_ext[h][:, c0:c1])
                    off += nblk
                ot_stage = ot_pool.tile([BLK, T], f32, tag="ot")
                sums_stage = ot_pool.tile([1, T], f32, tag="sums")

                seq_off = 0
                for nblk in seq_blocks:
                    Ls = nblk * BLK
                    n_groups = (Ls + GROUP - 1) // GROUP
                    # per-group accumulators live across the whole j loop
                    oT_ps = [
                        ps_o.tile([BLK, GROUP], f32, tag="ot_ps", name="oT_ps")
                        for _ in range(n_groups)
                    ]
                    sums_ps = [
                        ps_sum.tile([1, GROUP], f32, tag="sums_ps", name="sums_ps")
                        for _ in range(n_groups)
                    ]
                    g_started = [False] * n_groups
                    for j in range(nblk):
                        kj = kT_sb[:, seq_off + j * BLK : seq_off + (j + 1) * BLK]
                        vj = v_sb[:, seq_off // BLK + j, :]
                        for g in range(n_groups):
                            Q0 = g * GROUP
                            W = min(GROUP, Ls - Q0)
                            cs = max(0, BLK * j - Q0)
                            if cs >= W:
                                continue  # k block entirely after this group
                            N = W - cs
                            s_ps = ps_s.tile([BLK, GROUP], f32, tag="s_ps")
                            nc.tensor.matmul(
                                s_ps[:, :N],
                                kj,
                                qT_sb[:, seq_off + Q0 + cs : seq_off + Q0 + cs + N],
                                start=True,
                                stop=True,
                            )
                            if BLK * j >= Q0:  # diagonal block: causal mask
                                nc.vector.tensor_add(
                                    s_ps[:, :BLK], s_ps[:, :BLK], mask_sb[:]
                                )
                            p_b = p_pool.tile([BLK, GROUP], bf16, tag="p")
                            nc.scalar.activation(
                                p_b[:, :N],
                                s_ps[:, :N],
                                mybir.ActivationFunctionType.Exp,
                                scale=SCALE,
                            )
                            last = j == (Q0 + W) // BLK - 1
                            nc.tensor.matmul(
                                oT_ps[g][:, cs : cs + N],
                                vj,
                                p_b[:, :N],
                                start=not g_started[g],
                                stop=last,
                            )
                            nc.tensor.matmul(
                                sums_ps[g][:, cs : cs + N],
                                ones_b[:],
                                p_b[:, :N],
                                start=not g_started[g],
                                stop=last,
                            )
                            g_started[g] = True
                    for g in range(n_groups):
                        Q0 = g * GROUP
                        W = min(GROUP, Ls - Q0)
                        nc.vector.tensor_copy(
                            ot_stage[:, seq_off + Q0 : seq_off + Q0 + W],
                            oT_ps[g][:, :W],
                        )
                        nc.vector.tensor_copy(
                            sums_stage[:, seq_off + Q0 : seq_off + Q0 + W],
                            sums_ps[g][:, :W],
                        )
                    seq_off += Ls

                nc.sync.dma_start(oT_ext[h][:], ot_stage[:])
                nc.sync.dma_start(sums_ext[h][:], sums_stage[:])

    nc.finalize()
    return nc


def _install_ntff_hook():
    """Shim antenv.axon_hooks (absent in this container) so trace=True can
    reach the terminal's NRT profiler via libaxon_pjrt.so ctypes."""
    import types

    if "antenv.axon_hooks" in sys.modules:
        return
    import antenv
    from concourse import bass_utils

    mod = types.ModuleType("antenv.axon_hooks")
    state = {"hook": None}
    mod.set_axon_ntff_profile_hook = lambda h: state.__setitem__("hook", h)
    mod.get_axon_ntff_profile_hook = lambda: state["hook"]
    sys.modules["antenv.axon_hooks"] = mod
    antenv.axon_hooks = mod
    bass_utils.upload_artifacts = lambda tmpdir: tmpdir  # zero-egress container
    try:
        if "/root/.axon_site" not in sys.path:
            sys.path.insert(0, "/root/.axon_site")
        from trn_agent_boot.trn_boot import _ntff_profile_via_ctypes

        mod.set_axon_ntff_profile_hook(
            _ntff_profile_via_ctypes("/opt/axon/libaxon_pjrt.so")
        )
    except Exception:
        pass


def kernel(q, k, v, cu_seqlens, max_seqlen):
    from concourse import bass_utils

    q = np.asarray(q, dtype=np.float32)
    k = np.asarray(k, dtype=np.float32)
    v = np.asarray(v, dtype=np.float32)
    cu = np.asarray(cu_seqlens, dtype=np.int64)
    T_host = q.shape[0]
    lengths = np.diff(cu).astype(np.int64)
    nblocks = [int((L + BLK - 1) // BLK) for L in lengths]
    T_pad = sum(nblocks) * BLK

    # host -> padded device token index map (valid tokens only)
    dev_idx = np.zeros(T_host, dtype=np.int64)
    pad_off = 0
    for s, L in enumerate(lengths):
        L = int(L)
        dev_idx[cu[s] : cu[s] + L] = pad_off + np.arange(L)
        pad_off += nblocks[s] * BLK

    bf16 = ml_dtypes.bfloat16
    qp = np.zeros((T_pad, NUM_HEADS * HEAD_DIM), bf16)
    kp = np.zeros((T_pad, NUM_KV_HEADS * HEAD_DIM), bf16)
    vp = np.zeros((T_pad, NUM_KV_HEADS * HEAD_DIM), bf16)
    qp[dev_idx] = q.astype(bf16)
    kp[dev_idx] = k.astype(bf16)
    vp[dev_idx] = v.astype(bf16)

    mask = np.where(
        np.arange(BLK)[:, None] <= np.arange(BLK)[None, :], 0.0, -1e30
    ).astype(np.float32)

    key = tuple(nblocks)
    if key not in _GRAPH_CACHE:
        _GRAPH_CACHE[key] = _build_graph(key)
    nc = _GRAPH_CACHE[key]

    in_maps = []
    for c in range(N_CORES):
        m = {"mask": mask}
        m["kT"] = np.ascontiguousarray(kp[:, c * HEAD_DIM : (c + 1) * HEAD_DIM].T)
        m["v"] = np.ascontiguousarray(vp[:, c * HEAD_DIM : (c + 1) * HEAD_DIM])
        for h in range(HEADS_PER_CORE):
            gh = c * HEADS_PER_CORE + h
            m[f"qT{h}"] = np.ascontiguousarray(
                qp[:, gh * HEAD_DIM : (gh + 1) * HEAD_DIM].T
            )
        in_maps.append(m)

    trace = bool(os.environ.get("BASS_TRACE"))
    if trace:
        _install_ntff_hook()
    res = bass_utils.run_bass_kernel_spmd(
        nc, in_maps, core_ids=list(range(N_CORES)), trace=trace
    )
    if trace and res.exec_time_ns is not None:
        print(f"HW exec time: {res.exec_time_ns} ns")
        if res.instructions_and_trace is not None:
            print(f"trace: {res.instructions_and_trace[1]}")

    out = np.empty((T_host, NUM_HEADS * HEAD_DIM), np.float32)
    for c in range(N_CORES):
        r = res.results[c]
        for h in range(HEADS_PER_CORE):
            gh = c * HEADS_PER_CORE + h
            oT = r[f"oT{h}"]  # [128, T_pad] unnormalized
            sums = r[f"sums{h}"][0]  # [T_pad]
            o = (oT[:, dev_idx] / sums[dev_idx][None, :]).T  # [T_host, 128]
            out[:, gh * HEAD_DIM : (gh + 1) * HEAD_DIM] = o
    return out


# revision 14
# speedup vs baseline: 1.2380x; 1.0330x over previous
"""Varlen causal GQA attention on 8 TRN2 NeuronCores.

Problem: 32 q heads, 8 kv heads, head_dim 128, ragged batch (cu_seqlens),
f32. Sharded by KV-head group: core c owns kv head c and q heads
4c..4c+3 — fully data-independent across cores, no collectives.

Per core, for each of its 4 q heads, blockwise causal attention per
sequence with k-blocks in the outer loop (stationary reuse across the
q-groups of a sequence):
    S^T[k, q] = (K_j)^T.T @ Q^T          (bf16 matmul, d contracted)
    S^T += causal mask on diagonal block (DVE, fp32 in PSUM)
    P^T = exp(S^T * scale)               (ScalarE, bf16 out)
    O^T[d, q] += V_j @ P^T               (lhsT = V_j natural [k, d])
    sums[1, q] += ones.T @ P^T
Host does all transposes (Q^T/K^T in, O^T -> O out), the bf16 input
conversion, and the final softmax division, so the device executes only
matmuls, exp, mask adds, and PSUM->SBUF copies.
"""

import math
import os
import sys

sys.path.insert(0, "/opt/trn_rl_repo")

import ml_dtypes
import numpy as np

NUM_HEADS = 32
NUM_KV_HEADS = 8
HEAD_DIM = 128
HEADS_PER_CORE = NUM_HEADS // NUM_KV_HEADS  # 4
N_CORES = 8
BLK = 128
GROUP = 512
SCALE = 1.0 / math.sqrt(HEAD_DIM)

_GRAPH_CACHE = {}


def _build_graph(seq_blocks):
    """Build the SPMD Bacc graph for padded per-seq block counts."""
    from concourse import bacc
    import concourse.mybir as mybir
    from concourse.tile import TileContext

    f32 = mybir.dt.float32
    bf16 = mybir.dt.bfloat16
    T = sum(seq_blocks) * BLK
    n_blocks_total = T // BLK

    nc = bacc.Bacc("TRN2", target_bir_lowering=False, debug=False,
                   num_devices=N_CORES)

    qT_ext = [
        nc.declare_dram_parameter(f"qT{h}", [BLK, T], bf16, isOutput=False)
        for h in range(HEADS_PER_CORE)
    ]
    kT_ext = nc.declare_dram_parameter("kT", [BLK, T], bf16, isOutput=False)
    v_ext = nc.declare_dram_parameter("v", [T, HEAD_DIM], bf16, isOutput=False)
    mask_ext = nc.declare_dram_parameter("mask", [BLK, BLK], f32, isOutput=False)
    oT_ext = [
        nc.declare_dram_parameter(f"oT{h}", [BLK, T], f32, isOutput=True)
        for h in range(HEADS_PER_CORE)
    ]
    sums_ext = [
        nc.declare_dram_parameter(f"sums{h}", [1, T], f32, isOutput=True)
        for h in range(HEADS_PER_CORE)
    ]

    with TileContext(nc) as tc:
        with (
            tc.tile_pool(name="persist", bufs=1) as persist,
            tc.tile_pool(name="qr", bufs=2) as qr_pool,
            tc.tile_pool(name="ot", bufs=2) as ot_pool,
            tc.tile_pool(name="p", bufs=6) as p_pool,
            tc.tile_pool(name="ps_s", bufs=3, space="PSUM") as ps_s,
            tc.tile_pool(name="ps_o", bufs=3, space="PSUM") as ps_o,
            tc.tile_pool(name="ps_sum", bufs=2, space="PSUM") as ps_sum,
        ):
            kT_sb = persist.tile([BLK, T], bf16)
            v_sb = persist.tile([BLK, n_blocks_total, HEAD_DIM], bf16)
            mask_sb = persist.tile([BLK, BLK], f32)
            # first sequence's k/v + mask land first so compute starts early;
            # remaining sequences stream in one DMA each during compute
            v_re = v_ext[:].rearrange("(j p) d -> p j d", p=BLK)
            nb0 = seq_blocks[0]
            nc.sync.dma_start(kT_sb[:, : nb0 * BLK], kT_ext[:, : nb0 * BLK])
            nc.sync.dma_start(v_sb[:, :nb0, :], v_re[:, :nb0, :])
            nc.sync.dma_start(mask_sb[:], mask_ext[:])
            q_head_dma = []  # deferred per-head q loads, issued below
            if nb0 < n_blocks_total:
                c0 = nb0 * BLK
                nc.sync.dma_start(kT_sb[:, c0:], kT_ext[:, c0:])
                nc.sync.dma_start(v_sb[:, nb0:, :], v_re[:, nb0:, :])

            ones_f = persist.tile([BLK, 1], f32)
            nc.vector.memset(ones_f[:], 1.0)
            ones_b = persist.tile([BLK, 1], bf16)
            nc.vector.tensor_copy(ones_b[:], ones_f[:])

            for h in range(HEADS_PER_CORE):
                qT_sb = qr_pool.tile([BLK, T], bf16, tag="q")
                if h == 0:
                    # head 0: first sequence first, rest follows
                    nc.sync.dma_start(
                        qT_sb[:, : nb0 * BLK], qT_ext[h][:, : nb0 * BLK]
                    )
                    if nb0 < n_blocks_total:
                        nc.sync.dma_start(
                            qT_sb[:, nb0 * BLK :], qT_ext[h][:, nb0 * BLK :]
                        )
                else:
                    nc.sync.dma_start(qT_sb[:], qT_ext[h][:])
                ot_stage = ot_pool.tile([BLK, T], f32, tag="ot")
                sums_stage = ot_pool.tile([1, T], f32, tag="sums")

                seq_off = 0
                for nblk in seq_blocks:
                    Ls = nblk * BLK
                    n_groups = (Ls + GROUP - 1) // GROUP
                    # per-group accumulators live across the whole j loop
                    oT_ps = [
                        ps_o.tile([BLK, GROUP], f32, tag="ot_ps", name="oT_ps")
                        for _ in range(n_groups)
                    ]
                    sums_ps = [
                        ps_sum.tile([1, GROUP], f32, tag="sums_ps", name="sums_ps")
                        for _ in range(n_groups)
                    ]
                    g_started = [False] * n_groups
                    for j in range(nblk):
                        kj = kT_sb[:, seq_off + j * BLK : seq_off + (j + 1) * BLK]
                        vj = v_sb[:, seq_off // BLK + j, :]
                        for g in range(n_groups):
                            Q0 = g * GROUP
                            W = min(GROUP, Ls - Q0)
                            cs = max(0, BLK * j - Q0)
                            if cs >= W:
                                continue  # k block entirely after this group
                            N = W - cs
                            s_ps = ps_s.tile([BLK, GROUP], f32, tag="s_ps")
                            nc.tensor.matmul(
                                s_ps[:, :N],
                                kj,
                                qT_sb[:, seq_off + Q0 + cs : seq_off + Q0 + cs + N],
                                start=True,
                                stop=True,
                            )
                            if BLK * j >= Q0:  # diagonal block: causal mask
                                nc.vector.tensor_add(
                                    s_ps[:, :BLK], s_ps[:, :BLK], mask_sb[:]
                                )
                            p_b = p_pool.tile([BLK, GROUP], bf16, tag="p")
                            nc.scalar.activation(
                                p_b[:, :N],
                                s_ps[:, :N],
                                mybir.ActivationFunctionType.Exp,
                                scale=SCALE,
                            )
                            last = j == (Q0 + W) // BLK - 1
                            nc.tensor.matmul(
                                oT_ps[g][:, cs : cs + N],
                                vj,
                                p_b[:, :N],
                                start=not g_started[g],
                                stop=last,
                            )
                            nc.tensor.matmul(
                                sums_ps[g][:, cs : cs + N],
                                ones_b[:],
                                p_b[:, :N],
                                start=not g_started[g],
                                stop=last,
                            )
                            g_started[g] = True
                    for g in range(n_groups):
                        Q0 = g * GROUP
                        W = min(GROUP, Ls - Q0)
                        nc.vector.tensor_copy(
                            ot_stage[:, seq_off + Q0 : seq_off + Q0 + W],
                            oT_ps[g][:, :W],
                        )
                        nc.vector.tensor_copy(
                            sums_stage[:, seq_off + Q0 : seq_off + Q0 + W],
                            sums_ps[g][:, :W],
                        )
                    # stream this sequence's output while later seqs compute
                    nc.sync.dma_start(
                        oT_ext[h][:, seq_off : seq_off + Ls],
                        ot_stage[:, seq_off : seq_off + Ls],
                    )
                    seq_off += Ls

                nc.sync.dma_start(sums_ext[h][:], sums_stage[:])

    nc.finalize()
    return nc


def _install_ntff_hook():
    """Shim antenv.axon_hooks (absent in this container) so trace=True can
    reach the terminal's NRT profiler via libaxon_pjrt.so ctypes."""
    import types

    if "antenv.axon_hooks" in sys.modules:
        return
    import antenv
    from concourse import bass_utils

    mod = types.ModuleType("antenv.axon_hooks")
    state = {"hook": None}
    mod.set_axon_ntff_profile_hook = lambda h: state.__setitem__("hook", h)
    mod.get_axon_ntff_profile_hook = lambda: state["hook"]
    sys.modules["antenv.axon_hooks"] = mod
    antenv.axon_hooks = mod
    bass_utils.upload_artifacts = lambda tmpdir: tmpdir  # zero-egress container
    try:
        if "/root/.axon_site" not in sys.path:
            sys.path.insert(0, "/root/.axon_site")
        from trn_agent_boot.trn_boot import _ntff_profile_via_ctypes

        mod.set_axon_ntff_profile_hook(
            _ntff_profile_via_ctypes("/opt/axon/libaxon_pjrt.so")
        )
    except Exception:
        pass


def kernel(q, k, v, cu_seqlens, max_seqlen):
    from concourse import bass_utils

    q = np.asarray(q, dtype=np.float32)
    k = np.asarray(k, dtype=np.float32)
    v = np.asarray(v, dtype=np.float32)
    cu = np.asarray(cu_seqlens, dtype=np.int64)
    T_host = q.shape[0]
    lengths = np.diff(cu).astype(np.int64)
    nblocks = [int((L + BLK - 1) // BLK) for L in lengths]
    T_pad = sum(nblocks) * BLK

    # host -> padded device token index map (valid tokens only)
    dev_idx = np.zeros(T_host, dtype=np.int64)
    pad_off = 0
    for s, L in enumerate(lengths):
        L = int(L)
        dev_idx[cu[s] : cu[s] + L] = pad_off + np.arange(L)
        pad_off += nblocks[s] * BLK

    bf16 = ml_dtypes.bfloat16
    qp = np.zeros((T_pad, NUM_HEADS * HEAD_DIM), bf16)
    kp = np.zeros((T_pad, NUM_KV_HEADS * HEAD_DIM), bf16)
    vp = np.zeros((T_pad, NUM_KV_HEADS * HEAD_DIM), bf16)
    qp[dev_idx] = q.astype(bf16)
    kp[dev_idx] = k.astype(bf16)
    vp[dev_idx] = v.astype(bf16)

    mask = np.where(
        np.arange(BLK)[:, None] <= np.arange(BLK)[None, :], 0.0, -1e30
    ).astype(np.float32)

    key = tuple(nblocks)
    if key not in _GRAPH_CACHE:
        _GRAPH_CACHE[key] = _build_graph(key)
    nc = _GRAPH_CACHE[key]

    in_maps = []
    for c in range(N_CORES):
        m = {"mask": mask}
        m["kT"] = np.ascontiguousarray(kp[:, c * HEAD_DIM : (c + 1) * HEAD_DIM].T)
        m["v"] = np.ascontiguousarray(vp[:, c * HEAD_DIM : (c + 1) * HEAD_DIM])
        for h in range(HEADS_PER_CORE):
            gh = c * HEADS_PER_CORE + h
            m[f"qT{h}"] = np.ascontiguousarray(
                qp[:, gh * HEAD_DIM : (gh + 1) * HEAD_DIM].T
            )
        in_maps.append(m)

    trace = bool(os.environ.get("BASS_TRACE"))
    if trace:
        _install_ntff_hook()
    res = bass_utils.run_bass_kernel_spmd(
        nc, in_maps, core_ids=list(range(N_CORES)), trace=trace
    )
    if trace and res.exec_time_ns is not None:
        print(f"HW exec time: {res.exec_time_ns} ns")
        if res.instructions_and_trace is not None:
            print(f"trace: {res.instructions_and_trace[1]}")

    out = np.empty((T_host, NUM_HEADS * HEAD_DIM), np.float32)
    for c in range(N_CORES):
        r = res.results[c]
        for h in range(HEADS_PER_CORE):
            gh = c * HEADS_PER_CORE + h
            oT = r[f"oT{h}"]  # [128, T_pad] unnormalized
            sums = r[f"sums{h}"][0]  # [T_pad]
            o = (oT[:, dev_idx] / sums[dev_idx][None, :]).T  # [T_host, 128]
            out[:, gh * HEAD_DIM : (gh + 1) * HEAD_DIM] = o
    return out


# revision 18
# speedup vs baseline: 1.5153x; 1.2239x over previous
"""Varlen causal GQA attention on 8 TRN2 NeuronCores.

Problem: 32 q heads, 8 kv heads, head_dim 128, ragged batch (cu_seqlens),
f32. Sharded by KV-head group: core c owns kv head c and q heads
4c..4c+3 — fully data-independent across cores, no collectives.

Per core, for each of its 4 q heads, blockwise causal attention per
sequence with k-blocks in the outer loop (stationary reuse across the
q-groups of a sequence):
    S^T[k, q] = (K_j)^T.T @ Q^T          (bf16 matmul, d contracted)
    S^T += causal mask on diagonal block (DVE, fp32 in PSUM)
    P^T = exp(S^T * scale)               (ScalarE, bf16 out)
    O^T[d, q] += V_j @ P^T               (lhsT = V_j natural [k, d])
    sums[1, q] += ones.T @ P^T
Host does all transposes (Q^T/K^T in, O^T -> O out), the bf16 input
conversion, and the final softmax division, so the device executes only
matmuls, exp, mask adds, and PSUM->SBUF copies.
"""

import math
import os
import sys

sys.path.insert(0, "/opt/trn_rl_repo")

import ml_dtypes
import numpy as np

NUM_HEADS = 32
NUM_KV_HEADS = 8
HEAD_DIM = 128
HEADS_PER_CORE = NUM_HEADS // NUM_KV_HEADS  # 4
N_CORES = 8
BLK = 128
GROUP = 512
SCALE = 1.0 / math.sqrt(HEAD_DIM)

_GRAPH_CACHE = {}


def _build_graph(seq_blocks):
    """Build the SPMD Bacc graph for padded per-seq block counts."""
    from concourse import bacc
    import concourse.mybir as mybir
    from concourse.tile import TileContext

    f32 = mybir.dt.float32
    bf16 = mybir.dt.bfloat16
    T = sum(seq_blocks) * BLK
    n_blocks_total = T // BLK

    nc = bacc.Bacc("TRN2", target_bir_lowering=False, debug=False,
                   num_devices=N_CORES)

    qT_ext = [
        nc.declare_dram_parameter(f"qT{h}", [BLK, T], bf16, isOutput=False)
        for h in range(HEADS_PER_CORE)
    ]
    kT_ext = nc.declare_dram_parameter("kT", [BLK, T], bf16, isOutput=False)
    v_ext = nc.declare_dram_parameter("v", [T, HEAD_DIM], bf16, isOutput=False)
    mask_ext = nc.declare_dram_parameter("mask", [BLK, BLK], bf16, isOutput=False)
    oT_ext = [
        nc.declare_dram_parameter(f"oT{h}", [BLK, T], f32, isOutput=True)
        for h in range(HEADS_PER_CORE)
    ]
    sums_ext = [
        nc.declare_dram_parameter(f"sums{h}", [1, T], f32, isOutput=True)
        for h in range(HEADS_PER_CORE)
    ]

    with TileContext(nc) as tc:
        with (
            tc.tile_pool(name="persist", bufs=1) as persist,
            tc.tile_pool(name="qr", bufs=2) as qr_pool,
            tc.tile_pool(name="ot", bufs=2) as ot_pool,
            tc.tile_pool(name="p", bufs=6) as p_pool,
            tc.tile_pool(name="ps_s", bufs=3, space="PSUM") as ps_s,
            tc.tile_pool(name="ps_o", bufs=3, space="PSUM") as ps_o,
            tc.tile_pool(name="ps_sum", bufs=2, space="PSUM") as ps_sum,
        ):
            kT_sb = persist.tile([BLK, T], bf16)
            v_sb = persist.tile([BLK, n_blocks_total, HEAD_DIM], bf16)
            mask_sb = persist.tile([BLK, BLK], bf16)
            # first sequence's k/v (Sync queue) + q head 0 (Scalar queue, idle
            # during the prologue) land first so compute starts early; the
            # remaining data streams in during compute
            v_re = v_ext[:].rearrange("(j p) d -> p j d", p=BLK)
            nb0 = seq_blocks[0]
            nc.sync.dma_start(kT_sb[:, : nb0 * BLK], kT_ext[:, : nb0 * BLK])
            nc.sync.dma_start(v_sb[:, :nb0, :], v_re[:, :nb0, :])
            nc.scalar.dma_start(mask_sb[:], mask_ext[:])
            if nb0 < n_blocks_total:
                c0 = nb0 * BLK
                nc.sync.dma_start(kT_sb[:, c0:], kT_ext[:, c0:])
                nc.sync.dma_start(v_sb[:, nb0:, :], v_re[:, nb0:, :])

            ones_f = persist.tile([BLK, 1], f32)
            nc.vector.memset(ones_f[:], 1.0)
            ones_b = persist.tile([BLK, 1], bf16)
            nc.vector.tensor_copy(ones_b[:], ones_f[:])

            for h in range(HEADS_PER_CORE):
                qT_sb = qr_pool.tile([BLK, T], bf16, tag="q")
                if h == 0:
                    nc.scalar.dma_start(
                        qT_sb[:, : nb0 * BLK], qT_ext[h][:, : nb0 * BLK]
                    )
                    if nb0 < n_blocks_total:
                        nc.scalar.dma_start(
                            qT_sb[:, nb0 * BLK :], qT_ext[h][:, nb0 * BLK :]
                        )
                else:
                    nc.sync.dma_start(qT_sb[:], qT_ext[h][:])
                ot_stage = ot_pool.tile([BLK, T], f32, tag="ot")
                sums_stage = ot_pool.tile([1, T], f32, tag="sums")

                seq_off = 0
                for nblk in seq_blocks:
                    Ls = nblk * BLK
                    n_groups = (Ls + GROUP - 1) // GROUP
                    # per-group accumulators live across the whole j loop
                    oT_ps = [
                        ps_o.tile([BLK, GROUP], f32, tag="ot_ps", name="oT_ps")
                        for _ in range(n_groups)
                    ]
                    sums_ps = [
                        ps_sum.tile([1, GROUP], f32, tag="sums_ps", name="sums_ps")
                        for _ in range(n_groups)
                    ]
                    g_started = [False] * n_groups
                    for j in range(nblk):
                        kj = kT_sb[:, seq_off + j * BLK : seq_off + (j + 1) * BLK]
                        vj = v_sb[:, seq_off // BLK + j, :]
                        for g in range(n_groups):
                            Q0 = g * GROUP
                            W = min(GROUP, Ls - Q0)
                            cs = max(0, BLK * j - Q0)
                            if cs >= W:
                                continue  # k block entirely after this group
                            N = W - cs
                            s_ps = ps_s.tile([BLK, GROUP], f32, tag="s_ps")
                            nc.tensor.matmul(
                                s_ps[:, :N],
                                kj,
                                qT_sb[:, seq_off + Q0 + cs : seq_off + Q0 + cs + N],
                                start=True,
                                stop=True,
                            )
                            p_b = p_pool.tile([BLK, GROUP], bf16, tag="p")
                            nc.scalar.activation(
                                p_b[:, :N],
                                s_ps[:, :N],
                                mybir.ActivationFunctionType.Exp,
                                scale=SCALE,
                            )
                            if BLK * j >= Q0:
                                # causal mask: zero the upper triangle of the
                                # diagonal block post-exp, on idle GpSimd so
                                # the DVE/ACT queues stay out of this chain
                                nc.gpsimd.tensor_mul(
                                    p_b[:, :BLK], p_b[:, :BLK], mask_sb[:]
                                )
                            last = j == (Q0 + W) // BLK - 1
                            nc.tensor.matmul(
                                oT_ps[g][:, cs : cs + N],
                                vj,
                                p_b[:, :N],
                                start=not g_started[g],
                                stop=last,
                            )
                            nc.tensor.matmul(
                                sums_ps[g][:, cs : cs + N],
                                ones_b[:],
                                p_b[:, :N],
                                start=not g_started[g],
                                stop=last,
                            )
                            g_started[g] = True
                    for g in range(n_groups):
                        Q0 = g * GROUP
                        W = min(GROUP, Ls - Q0)
                        nc.vector.tensor_copy(
                            ot_stage[:, seq_off + Q0 : seq_off + Q0 + W],
                            oT_ps[g][:, :W],
                        )
                        nc.vector.tensor_copy(
                            sums_stage[:, seq_off + Q0 : seq_off + Q0 + W],
                            sums_ps[g][:, :W],
                        )
                    # stream this sequence's output while later seqs compute
                    nc.sync.dma_start(
                        oT_ext[h][:, seq_off : seq_off + Ls],
                        ot_stage[:, seq_off : seq_off + Ls],
                    )
                    seq_off += Ls

                nc.sync.dma_start(sums_ext[h][:], sums_stage[:])

    nc.finalize()
    return nc


def _install_ntff_hook():
    """Shim antenv.axon_hooks (absent in this container) so trace=True can
    reach the terminal's NRT profiler via libaxon_pjrt.so ctypes."""
    import types

    if "antenv.axon_hooks" in sys.modules:
        return
    import antenv
    from concourse import bass_utils

    mod = types.ModuleType("antenv.axon_hooks")
    state = {"hook": None}
    mod.set_axon_ntff_profile_hook = lambda h: state.__setitem__("hook", h)
    mod.get_axon_ntff_profile_hook = lambda: state["hook"]
    sys.modules["antenv.axon_hooks"] = mod
    antenv.axon_hooks = mod
    bass_utils.upload_artifacts = lambda tmpdir: tmpdir  # zero-egress container
    try:
        if "/root/.axon_site" not in sys.path:
            sys.path.insert(0, "/root/.axon_site")
        from trn_agent_boot.trn_boot import _ntff_profile_via_ctypes

        mod.set_axon_ntff_profile_hook(
            _ntff_profile_via_ctypes("/opt/axon/libaxon_pjrt.so")
        )
    except Exception:
        pass


def kernel(q, k, v, cu_seqlens, max_seqlen):
    from concourse import bass_utils

    q = np.asarray(q, dtype=np.float32)
    k = np.asarray(k, dtype=np.float32)
    v = np.asarray(v, dtype=np.float32)
    cu = np.asarray(cu_seqlens, dtype=np.int64)
    T_host = q.shape[0]
    lengths = np.diff(cu).astype(np.int64)
    nblocks = [int((L + BLK - 1) // BLK) for L in lengths]
    T_pad = sum(nblocks) * BLK

    # host -> padded device token index map (valid tokens only)
    dev_idx = np.zeros(T_host, dtype=np.int64)
    pad_off = 0
    for s, L in enumerate(lengths):
        L = int(L)
        dev_idx[cu[s] : cu[s] + L] = pad_off + np.arange(L)
        pad_off += nblocks[s] * BLK

    bf16 = ml_dtypes.bfloat16
    qp = np.zeros((T_pad, NUM_HEADS * HEAD_DIM), bf16)
    kp = np.zeros((T_pad, NUM_KV_HEADS * HEAD_DIM), bf16)
    vp = np.zeros((T_pad, NUM_KV_HEADS * HEAD_DIM), bf16)
    qp[dev_idx] = q.astype(bf16)
    kp[dev_idx] = k.astype(bf16)
    vp[dev_idx] = v.astype(bf16)

    mask = np.where(
        np.arange(BLK)[:, None] <= np.arange(BLK)[None, :], 1.0, 0.0
    ).astype(bf16)

    key = tuple(nblocks)
    if key not in _GRAPH_CACHE:
        _GRAPH_CACHE[key] = _build_graph(key)
    nc = _GRAPH_CACHE[key]

    in_maps = []
    for c in range(N_CORES):
        m = {"mask": mask}
        m["kT"] = np.ascontiguousarray(kp[:, c * HEAD_DIM : (c + 1) * HEAD_DIM].T)
        m["v"] = np.ascontiguousarray(vp[:, c * HEAD_DIM : (c + 1) * HEAD_DIM])
        for h in range(HEADS_PER_CORE):
            gh = c * HEADS_PER_CORE + h
            m[f"qT{h}"] = np.ascontiguousarray(
                qp[:, gh * HEAD_DIM : (gh + 1) * HEAD_DIM].T
            )
        in_maps.append(m)

    trace = bool(os.environ.get("BASS_TRACE"))
    if trace:
        _install_ntff_hook()
    res = bass_utils.run_bass_kernel_spmd(
        nc, in_maps, core_ids=list(range(N_CORES)), trace=trace
    )
    if trace and res.exec_time_ns is not None:
        print(f"HW exec time: {res.exec_time_ns} ns")
        if res.instructions_and_trace is not None:
            print(f"trace: {res.instructions_and_trace[1]}")

    out = np.empty((T_host, NUM_HEADS * HEAD_DIM), np.float32)
    for c in range(N_CORES):
        r = res.results[c]
        for h in range(HEADS_PER_CORE):
            gh = c * HEADS_PER_CORE + h
            oT = r[f"oT{h}"]  # [128, T_pad] unnormalized
            sums = r[f"sums{h}"][0]  # [T_pad]
            o = (oT[:, dev_idx] / sums[dev_idx][None, :]).T  # [T_host, 128]
            out[:, gh * HEAD_DIM : (gh + 1) * HEAD_DIM] = o
    return out


# revision 22
# speedup vs baseline: 1.5283x; 1.0086x over previous
"""Varlen causal GQA attention on 8 TRN2 NeuronCores.

Problem: 32 q heads, 8 kv heads, head_dim 128, ragged batch (cu_seqlens),
f32. Sharded by KV-head group: core c owns kv head c and q heads
4c..4c+3 — fully data-independent across cores, no collectives.

Per core, for each of its 4 q heads, blockwise causal attention per
sequence with k-blocks in the outer loop (stationary reuse across the
q-groups of a sequence):
    S^T[k, q] = (K_j)^T.T @ Q^T          (bf16 matmul, d contracted)
    S^T += causal mask on diagonal block (DVE, fp32 in PSUM)
    P^T = exp(S^T * scale)               (ScalarE, bf16 out)
    O^T[d, q] += V_j @ P^T               (lhsT = V_j natural [k, d])
    sums[1, q] += ones.T @ P^T
Host does all transposes (Q^T/K^T in, O^T -> O out), the bf16 input
conversion, and the final softmax division, so the device executes only
matmuls, exp, mask adds, and PSUM->SBUF copies.
"""

import math
import os
import sys

sys.path.insert(0, "/opt/trn_rl_repo")

import ml_dtypes
import numpy as np

NUM_HEADS = 32
NUM_KV_HEADS = 8
HEAD_DIM = 128
HEADS_PER_CORE = NUM_HEADS // NUM_KV_HEADS  # 4
N_CORES = 8
BLK = 128
GROUP = 512
SCALE = 1.0 / math.sqrt(HEAD_DIM)

_GRAPH_CACHE = {}


def _build_graph(seq_blocks):
    """Build the SPMD Bacc graph for padded per-seq block counts."""
    from concourse import bacc
    import concourse.mybir as mybir
    from concourse.tile import TileContext

    f32 = mybir.dt.float32
    bf16 = mybir.dt.bfloat16
    T = sum(seq_blocks) * BLK
    n_blocks_total = T // BLK

    nc = bacc.Bacc("TRN2", target_bir_lowering=False, debug=False,
                   num_devices=N_CORES)

    qT_ext = [
        nc.declare_dram_parameter(f"qT{h}", [BLK, T], bf16, isOutput=False)
        for h in range(HEADS_PER_CORE)
    ]
    kT_ext = nc.declare_dram_parameter("kT", [BLK, T], bf16, isOutput=False)
    v_ext = nc.declare_dram_parameter("v", [T, HEAD_DIM], bf16, isOutput=False)
    mask_ext = nc.declare_dram_parameter("mask", [BLK, BLK], bf16, isOutput=False)
    oT_ext = [
        nc.declare_dram_parameter(f"oT{h}", [BLK, T], f32, isOutput=True)
        for h in range(HEADS_PER_CORE)
    ]
    sums_ext = [
        nc.declare_dram_parameter(f"sums{h}", [1, T], f32, isOutput=True)
        for h in range(HEADS_PER_CORE)
    ]

    with TileContext(nc) as tc:
        with (
            tc.tile_pool(name="persist", bufs=1) as persist,
            tc.tile_pool(name="qr", bufs=2) as qr_pool,
            tc.tile_pool(name="ot", bufs=2) as ot_pool,
            tc.tile_pool(name="p", bufs=6) as p_pool,
            tc.tile_pool(name="ps_s", bufs=3, space="PSUM") as ps_s,
            tc.tile_pool(name="ps_o", bufs=3, space="PSUM") as ps_o,
            tc.tile_pool(name="ps_sum", bufs=2, space="PSUM") as ps_sum,
        ):
            kT_sb = persist.tile([BLK, T], bf16)
            v_sb = persist.tile([BLK, n_blocks_total, HEAD_DIM], bf16)
            mask_sb = persist.tile([BLK, BLK], bf16)
            # first sequence's k/v (Sync queue) + q head 0 (Scalar queue, idle
            # during the prologue) land first so compute starts early; the
            # remaining data streams in during compute
            v_re = v_ext[:].rearrange("(j p) d -> p j d", p=BLK)
            nb0 = seq_blocks[0]
            nc.sync.dma_start(kT_sb[:, : nb0 * BLK], kT_ext[:, : nb0 * BLK])
            q0_first = None  # first q chunk issued here, right behind kT
            nc.sync.dma_start(v_sb[:, :nb0, :], v_re[:, :nb0, :])
            nc.scalar.dma_start(mask_sb[:], mask_ext[:])
            if nb0 < n_blocks_total:
                c0 = nb0 * BLK
                nc.scalar.dma_start(kT_sb[:, c0:], kT_ext[:, c0:])
                nc.scalar.dma_start(v_sb[:, nb0:, :], v_re[:, nb0:, :])

            ones_f = persist.tile([BLK, 1], f32)
            nc.vector.memset(ones_f[:], 1.0)
            ones_b = persist.tile([BLK, 1], bf16)
            nc.vector.tensor_copy(ones_b[:], ones_f[:])

            for h in range(HEADS_PER_CORE):
                qT_sb = qr_pool.tile([BLK, T], bf16, tag="q")
                if h == 0:
                    nc.sync.dma_start(
                        qT_sb[:, : nb0 * BLK], qT_ext[h][:, : nb0 * BLK]
                    )
                    if nb0 < n_blocks_total:
                        nc.sync.dma_start(
                            qT_sb[:, nb0 * BLK :], qT_ext[h][:, nb0 * BLK :]
                        )
                else:
                    nc.sync.dma_start(qT_sb[:], qT_ext[h][:])
                ot_stage = ot_pool.tile([BLK, T], f32, tag="ot")
                sums_stage = ot_pool.tile([1, T], f32, tag="sums")

                seq_off = 0
                for nblk in seq_blocks:
                    Ls = nblk * BLK
                    n_groups = (Ls + GROUP - 1) // GROUP
                    # per-group accumulators live across the whole j loop
                    oT_ps = [
                        ps_o.tile([BLK, GROUP], f32, tag="ot_ps", name="oT_ps")
                        for _ in range(n_groups)
                    ]
                    sums_ps = [
                        ps_sum.tile([1, GROUP], f32, tag="sums_ps", name="sums_ps")
                        for _ in range(n_groups)
                    ]
                    g_started = [False] * n_groups
                    for j in range(nblk):
                        kj = kT_sb[:, seq_off + j * BLK : seq_off + (j + 1) * BLK]
                        vj = v_sb[:, seq_off // BLK + j, :]
                        work = []  # (g, cs, N) for this k block
                        for g in range(n_groups):
                            Q0 = g * GROUP
                            W = min(GROUP, Ls - Q0)
                            cs = max(0, BLK * j - Q0)
                            if cs < W:
                                work.append((g, Q0, W, cs, W - cs))
                        # all S matmuls first (one K_j weight load), then exp,
                        # then AV (one V_j load), then sums (one ones load)
                        s_tiles, p_tiles = [], []
                        for g, Q0, W, cs, N in work:
                            s_ps = ps_s.tile([BLK, GROUP], f32, tag="s_ps")
                            nc.tensor.matmul(
                                s_ps[:, :N],
                                kj,
                                qT_sb[:, seq_off + Q0 + cs : seq_off + Q0 + cs + N],
                                start=True,
                                stop=True,
                            )
                            s_tiles.append(s_ps)
                        for (g, Q0, W, cs, N), s_ps in zip(work, s_tiles):
                            p_b = p_pool.tile([BLK, GROUP], bf16, tag="p")
                            nc.scalar.activation(
                                p_b[:, :N],
                                s_ps[:, :N],
                                mybir.ActivationFunctionType.Exp,
                                scale=SCALE,
                            )
                            if BLK * j >= Q0:
                                # causal mask: zero the upper triangle of the
                                # diagonal block post-exp, on idle GpSimd so
                                # the DVE/ACT queues stay out of this chain
                                nc.gpsimd.tensor_mul(
                                    p_b[:, :BLK], p_b[:, :BLK], mask_sb[:]
                                )
                            p_tiles.append(p_b)
                        for (g, Q0, W, cs, N), p_b in zip(work, p_tiles):
                            nc.tensor.matmul(
                                oT_ps[g][:, cs : cs + N],
                                vj,
                                p_b[:, :N],
                                start=not g_started[g],
                                stop=j == (Q0 + W) // BLK - 1,
                            )
                        for (g, Q0, W, cs, N), p_b in zip(work, p_tiles):
                            nc.tensor.matmul(
                                sums_ps[g][:, cs : cs + N],
                                ones_b[:],
                                p_b[:, :N],
                                start=not g_started[g],
                                stop=j == (Q0 + W) // BLK - 1,
                            )
                            g_started[g] = True
                    for g in range(n_groups):
                        Q0 = g * GROUP
                        W = min(GROUP, Ls - Q0)
                        nc.vector.tensor_copy(
                            ot_stage[:, seq_off + Q0 : seq_off + Q0 + W],
                            oT_ps[g][:, :W],
                        )
                        nc.vector.tensor_copy(
                            sums_stage[:, seq_off + Q0 : seq_off + Q0 + W],
                            sums_ps[g][:, :W],
                        )
                    # stream this sequence's output while later seqs compute
                    nc.sync.dma_start(
                        oT_ext[h][:, seq_off : seq_off + Ls],
                        ot_stage[:, seq_off : seq_off + Ls],
                    )
                    seq_off += Ls

                nc.sync.dma_start(sums_ext[h][:], sums_stage[:])

    nc.finalize()
    return nc


def _install_ntff_hook():
    """Shim antenv.axon_hooks (absent in this container) so trace=True can
    reach the terminal's NRT profiler via libaxon_pjrt.so ctypes."""
    import types

    if "antenv.axon_hooks" in sys.modules:
        return
    import antenv
    from concourse import bass_utils

    mod = types.ModuleType("antenv.axon_hooks")
    state = {"hook": None}
    mod.set_axon_ntff_profile_hook = lambda h: state.__setitem__("hook", h)
    mod.get_axon_ntff_profile_hook = lambda: state["hook"]
    sys.modules["antenv.axon_hooks"] = mod
    antenv.axon_hooks = mod
    bass_utils.upload_artifacts = lambda tmpdir: tmpdir  # zero-egress container
    try:
        if "/root/.axon_site" not in sys.path:
            sys.path.insert(0, "/root/.axon_site")
        from trn_agent_boot.trn_boot import _ntff_profile_via_ctypes

        mod.set_axon_ntff_profile_hook(
            _ntff_profile_via_ctypes("/opt/axon/libaxon_pjrt.so")
        )
    except Exception:
        pass


def kernel(q, k, v, cu_seqlens, max_seqlen):
    from concourse import bass_utils

    q = np.asarray(q, dtype=np.float32)
    k = np.asarray(k, dtype=np.float32)
    v = np.asarray(v, dtype=np.float32)
    cu = np.asarray(cu_seqlens, dtype=np.int64)
    T_host = q.shape[0]
    lengths = np.diff(cu).astype(np.int64)
    all_nblocks = [int((L + BLK - 1) // BLK) for L in lengths]
    T_pad = sum(all_nblocks) * BLK

    # process sequences longest-first: big seq warms the pipeline while the
    # rest of the data streams in, and the tail drains a small seq
    order = sorted(range(len(lengths)), key=lambda s: -all_nblocks[s])
    nblocks = [all_nblocks[s] for s in order]

    # host -> padded device token index map (valid tokens only)
    dev_idx = np.zeros(T_host, dtype=np.int64)
    pad_off = 0
    for s in order:
        L = int(lengths[s])
        dev_idx[cu[s] : cu[s] + L] = pad_off + np.arange(L)
        pad_off += all_nblocks[s] * BLK

    bf16 = ml_dtypes.bfloat16
    qp = np.zeros((T_pad, NUM_HEADS * HEAD_DIM), bf16)
    kp = np.zeros((T_pad, NUM_KV_HEADS * HEAD_DIM), bf16)
    vp = np.zeros((T_pad, NUM_KV_HEADS * HEAD_DIM), bf16)
    qp[dev_idx] = q.astype(bf16)
    kp[dev_idx] = k.astype(bf16)
    vp[dev_idx] = v.astype(bf16)

    mask = np.where(
        np.arange(BLK)[:, None] <= np.arange(BLK)[None, :], 1.0, 0.0
    ).astype(bf16)

    key = tuple(nblocks)
    if key not in _GRAPH_CACHE:
        _GRAPH_CACHE[key] = _build_graph(key)
    nc = _GRAPH_CACHE[key]

    in_maps = []
    for c in range(N_CORES):
        m = {"mask": mask}
        m["kT"] = np.ascontiguousarray(kp[:, c * HEAD_DIM : (c + 1) * HEAD_DIM].T)
        m["v"] = np.ascontiguousarray(vp[:, c * HEAD_DIM : (c + 1) * HEAD_DIM])
        for h in range(HEADS_PER_CORE):
            gh = c * HEADS_PER_CORE + h
            m[f"qT{h}"] = np.ascontiguousarray(
                qp[:, gh * HEAD_DIM : (gh + 1) * HEAD_DIM].T
            )
        in_maps.append(m)

    trace = bool(os.environ.get("BASS_TRACE"))
    if trace:
        _install_ntff_hook()
    res = bass_utils.run_bass_kernel_spmd(
        nc, in_maps, core_ids=list(range(N_CORES)), trace=trace
    )
    if trace and res.exec_time_ns is not None:
        print(f"HW exec time: {res.exec_time_ns} ns")
        if res.instructions_and_trace is not None:
            print(f"trace: {res.instructions_and_trace[1]}")

    out = np.empty((T_host, NUM_HEADS * HEAD_DIM), np.float32)
    for c in range(N_CORES):
        r = res.results[c]
        for h in range(HEADS_PER_CORE):
            gh = c * HEADS_PER_CORE + h
            oT = r[f"oT{h}"]  # [128, T_pad] unnormalized
            sums = r[f"sums{h}"][0]  # [T_pad]
            o = (oT[:, dev_idx] / sums[dev_idx][None, :]).T  # [T_host, 128]
            out[:, gh * HEAD_DIM : (gh + 1) * HEAD_DIM] = o
    return out
